# revision 4
# baseline (speedup 1.0000x reference)
"""Trainium2 Bass kernel for a 4-behavior GCN layer (u/i SpMM + heads).

Strategy (8 NeuronCores, SPMD):
  - A-spmm  (u_b = A_b @ item_side):  users (rows) sharded across cores.
  - AT-spmm (i_b = A_b^T @ user_side): items (cols) sharded across cores.
  - Edge lists sharded + sorted by destination; per 128-dst tile the source
    rows are fetched with dma_gather and reduced with a one-hot segment
    matmul on the tensor engine (scatter-free segment_sum).
  - Per-(behavior,dim) sum-of-squares accumulated on PSUM, AllReduce'd
    across the 8 cores for the global L2 normalization over the node axis.
  - mean -> @W -> sigmoid computed on-chip per 128-row tile.
"""
import os
import sys

sys.path.insert(0, "/opt/trn_rl_repo")

import numpy as np

NU, NI, D, NNZ = 80000, 40000, 128, 2000000
NCORES = 8
USH, ISH = NU // NCORES, NI // NCORES  # per-core shards: 10000 users, 5000 items
EPS2 = 1e-24  # max(norm,1e-12) == sqrt(max(ssq,1e-24)) for our value ranges
CALL_SLOTS = 1024  # max gather idxs per dma_gather call (SWDGE ring safety)
GW = 64  # one-hot group width (dsts per matmul)


def _ceil(a, b):
    return -(-a // b)


def _prep_side(dst, src, val, n_dst_shard, n_src):
    """Sort one core's edges of one (behavior, side) by (tile, range, group, src).

    Returns sorted columns + counts[T, R, 2] (edges per tile/src-range/64-group).
    """
    T = _ceil(n_dst_shard, 128)
    R = _ceil(n_src, 32768)
    tile_id = dst >> 7
    grp = (dst >> 6) & 1
    rng = src >> 15
    order = np.lexsort((src, grp, rng, tile_id))
    dst, src, val, tile_id, grp, rng = (
        a[order] for a in (dst, src, val, tile_id, grp, rng)
    )
    key = (tile_id * R + rng) * 2 + grp
    counts = np.bincount(key, minlength=T * R * 2).reshape(T, R, 2)
    return dst, src, val, counts


def _layout_side(pad, T, R):
    """From shared padded counts pad[T,R,2] (multiples of 128) build the static
    call/chunk layout for one (behavior, side). Returns per-tile:
      calls: list of (slot_off_in_tile, n_slots, range)
      chunks: list of (grp, start, stop) per 128-slot chunk
      tile_slots: slots per tile
    """
    tiles = []
    for t in range(T):
        calls = []
        chunks_g = []
        off = 0
        for r in range(R):
            s_r = int(pad[t, r, 0] + pad[t, r, 1])
            # call split within (tile, range)
            o = 0
            while o < s_r:
                n = min(CALL_SLOTS, s_r - o)
                calls.append((off + o, n, r))
                o += n
            for g in (0, 1):
                chunks_g.extend([g] * (int(pad[t, r, g]) // 128))
            off += s_r
        # start/stop flags per group
        first = {0: None, 1: None}
        last = {0: None, 1: None}
        for k, g in enumerate(chunks_g):
            if first[g] is None:
                first[g] = k
            last[g] = k
        chunks = [
            (g, k == first[g], k == last[g]) for k, g in enumerate(chunks_g)
        ]
        tiles.append((calls, chunks, off))
    return tiles


def _fill_arrays(dst, src, val, counts, pad, T, R, n_dst_shard):
    """Place this core's sorted edges into the shared padded slot layout.

    Returns idx16 [128, total_slots//16], dloc f32 [128, total_chunks],
    vals f32 [128, total_chunks], and per-tile slot offsets.
    """
    total_slots = int(pad.sum())
    total_chunks = total_slots // 128
    slot_idx = np.zeros(total_slots, np.int16)
    slot_dloc = np.zeros(total_slots, np.float32)
    slot_val = np.zeros(total_slots, np.float32)

    # cumulative source offsets of each (t, r, g) segment in the padded layout
    seg_pad = pad.reshape(-1)  # [T*R*2]
    seg_off = np.concatenate([[0], np.cumsum(seg_pad)])[:-1]
    # edges are sorted in exactly (t, r, g) segment order
    seg_cnt = counts.reshape(-1)
    e_off = np.concatenate([[0], np.cumsum(seg_cnt)])[:-1]
    # scatter each segment's edges to its padded slot range
    pos_in_seg = np.arange(len(dst)) - np.repeat(e_off, seg_cnt)
    slot_of_edge = np.repeat(seg_off, seg_cnt) + pos_in_seg
    rng_of_edge = np.repeat(np.arange(T * R * 2) // 2 % R, seg_cnt)
    slot_idx[slot_of_edge] = (src - (rng_of_edge << 15)).astype(np.int16)
    slot_dloc[slot_of_edge] = (dst & 63).astype(np.float32)
    slot_val[slot_of_edge] = val

    # idx16 wrap: per tile, per call, slot j -> [j%16, j//16], tiled x8
    tile_slots = pad.reshape(T, -1).sum(axis=1).astype(np.int64)
    tile_off = np.concatenate([[0], np.cumsum(tile_slots)])[:-1]
    idx16 = slot_idx.reshape(-1, 16).T.copy()  # [16, total/16] global wrap
    # NOTE: wrap must be per-call; calls are multiples of 128 slots so a
    # global 16-wrap restarts cleanly at every 128-slot boundary. Each call
    # starts at a multiple of 128 slots -> its own 16-wrap is just the
    # corresponding column range of the global wrap.
    idx16 = np.tile(idx16, (8, 1))  # replicate to 128 partitions

    dloc = slot_dloc.reshape(-1, 128).T.copy()  # [128, total_chunks]
    vals = slot_val.reshape(-1, 128).T.copy()
    return idx16, dloc, vals, tile_off, total_slots, total_chunks


def _host_prep(inputs):
    """Build per-core input arrays + the shared static layout."""
    sides = []  # (side_key, n_dst_shard, n_src, T, R)
    # A-side: dst=users(rows), src=items(cols); AT-side: dst=items, src=users
    for b in range(4):
        sides.append(("u", b))
    for b in range(4):
        sides.append(("i", b))

    per_core = [dict() for _ in range(NCORES)]
    layouts = {}
    for side, b in sides:
        rows = np.asarray(inputs[f"rows{b}"])
        cols = np.asarray(inputs[f"cols{b}"])
        vals = np.asarray(inputs[f"vals{b}"])
        if side == "u":
            dsts, srcs, nds, nsrc = rows, cols, USH, NI
        else:
            dsts, srcs, nds, nsrc = cols, rows, ISH, NU
        T, R = _ceil(nds, 128), _ceil(nsrc, 32768)
        core_of = dsts // nds
        datas = []
        counts_all = []
        for c in range(NCORES):
            m = core_of == c
            d, s, v, cnt = _prep_side(
                (dsts[m] - c * nds).astype(np.int64),
                srcs[m].astype(np.int64),
                vals[m],
                nds,
                nsrc,
            )
            datas.append((d, s, v, cnt))
            counts_all.append(cnt)
        cmax = np.maximum.reduce(counts_all)
        pad = _ceil(cmax, 128) * 128
        pad[:, 0, :] = np.maximum(pad[:, 0, :], 128)  # >=1 chunk per (t,g)
        layouts[(side, b)] = (pad, T, R)
        for c in range(NCORES):
            d, s, v, cnt = datas[c]
            per_core[c][(side, b)] = _fill_arrays(d, s, v, cnt, pad, T, R, nds)
    return per_core, layouts


def _build_bass(layouts):
    import concourse.bass as bass
    import concourse.tile as tile
    from concourse import bacc, mybir

    f32 = mybir.dt.float32
    i16 = mybir.dt.int16

    nc = bacc.Bacc("TRN2", target_bir_lowering=False, debug=False,
                   num_devices=NCORES)

    # ---- dram tensors ----
    tabs = {}
    for b, nm in enumerate(["ii_embed0", "ii_embed1", "ii_embed2", "item_embedding"]):
        tabs[("u", b)] = nc.dram_tensor(nm, [NI, D], f32, kind="ExternalInput")
    for b, nm in enumerate(["uu_embed0", "uu_embed1", "uu_embed2", "user_embedding"]):
        tabs[("i", b)] = nc.dram_tensor(nm, [NU, D], f32, kind="ExternalInput")
    u_w = nc.dram_tensor("u_w", [D, D], f32, kind="ExternalInput")
    i_w = nc.dram_tensor("i_w", [D, D], f32, kind="ExternalInput")
    iota_in = nc.dram_tensor("iota64", [128, GW], f32, kind="ExternalInput")

    meta = {}
    for (side, b), (pad, T, R) in layouts.items():
        ts = int(pad.sum())
        tc_ = ts // 128
        meta[(side, b)] = dict(
            idx=nc.dram_tensor(f"idx_{side}{b}", [128, ts // 16], i16,
                               kind="ExternalInput"),
            dloc=nc.dram_tensor(f"dloc_{side}{b}", [128, tc_], f32,
                                kind="ExternalInput"),
            val=nc.dram_tensor(f"val_{side}{b}", [128, tc_], f32,
                               kind="ExternalInput"),
        )

    outs = {}
    for b in range(4):
        outs[("u", b)] = nc.dram_tensor(f"u{b}", [USH, D], f32, kind="ExternalOutput")
        outs[("i", b)] = nc.dram_tensor(f"i{b}", [ISH, D], f32, kind="ExternalOutput")
        outs[("lu", b)] = nc.dram_tensor(f"lu{b}", [USH, D], f32, kind="ExternalOutput")
        outs[("li", b)] = nc.dram_tensor(f"li{b}", [ISH, D], f32, kind="ExternalOutput")
    outs["user_out"] = nc.dram_tensor("user_out", [USH, D], f32, kind="ExternalOutput")
    outs["item_out"] = nc.dram_tensor("item_out", [ISH, D], f32, kind="ExternalOutput")

    cc_in = nc.dram_tensor("cc_in", [128, 8], f32)
    cc_out = nc.dram_tensor("cc_out", [128, 8], f32, addr_space="Shared")

    TU, TI = _ceil(USH, 128), _ceil(ISH, 128)

    with tile.TileContext(nc) as tc:
        import contextlib

        with contextlib.ExitStack() as ctx:
            const_p = ctx.enter_context(tc.tile_pool(name="const", bufs=1))
            slab_p = ctx.enter_context(tc.tile_pool(name="slab", bufs=1))

            iota = const_p.tile([128, GW], f32)
            nc.sync.dma_start(iota[:], iota_in.ap()[:, :])
            ones_col = const_p.tile([128, 1], f32)
            nc.vector.memset(ones_col[:], 1.0)
            ones_row = const_p.tile([1, 128], f32)
            nc.vector.memset(ones_row[:], 1.0)
            uw_sb = const_p.tile([D, D], f32)
            nc.sync.dma_start(uw_sb[:], u_w.ap()[:, :])
            iw_sb = const_p.tile([D, D], f32)
            nc.sync.dma_start(iw_sb[:], i_w.ap()[:, :])

            mean_u = slab_p.tile([128, TU * 128], f32)
            nc.vector.memset(mean_u[:], 0.0)
            mean_i = slab_p.tile([128, TI * 128], f32)
            nc.vector.memset(mean_i[:], 0.0)

            # ---------- phase 1: the 8 spmm sides ----------
            with tc.tile_pool(name="io", bufs=3) as io_p, \
                 tc.tile_pool(name="g", bufs=5) as g_p, \
                 tc.tile_pool(name="m", bufs=2) as m_p, \
                 tc.tile_pool(name="ev", bufs=3) as ev_p, \
                 tc.tile_pool(name="ps", bufs=2, space="PSUM") as ps_p, \
                 tc.tile_pool(name="pssq", bufs=1, space="PSUM") as pssq_p:

                ssq = pssq_p.tile([128, 8], f32)

                for si, (side, nds, T, mean_sl) in enumerate(
                    [("u", USH, TU, mean_u), ("i", ISH, TI, mean_i)]
                ):
                    for b in range(4):
                        pad, T_, R = layouts[(side, b)]
                        tiles = _layout_side(pad, T_, R)
                        mt = meta[(side, b)]
                        table = tabs[(side, b)]
                        n_src = NI if side == "u" else NU
                        ssq_col = b + 4 * si
                        # running offsets into the meta tensors
                        slot_off = 0
                        chunk_off = 0
                        for t in range(T_):
                            calls, chunks, tile_slots = tiles[t]
                            n_ch = tile_slots // 128
                            isb = io_p.tile([128, max(tile_slots // 16, 8)], i16,
                                            tag="isb")
                            nc.sync.dma_start(
                                isb[:, : tile_slots // 16],
                                mt["idx"].ap()[:, slot_off // 16:
                                               (slot_off + tile_slots) // 16])
                            dl = io_p.tile([128, max(n_ch, 8)], f32, tag="dl")
                            nc.sync.dma_start(
                                dl[:, :n_ch],
                                mt["dloc"].ap()[:, chunk_off:chunk_off + n_ch])
                            vl = io_p.tile([128, max(n_ch, 8)], f32, tag="vl")
                            nc.sync.dma_start(
                                vl[:, :n_ch],
                                mt["val"].ap()[:, chunk_off:chunk_off + n_ch])

                            # one-hot M for all chunks of this tile
                            mtile = m_p.tile([128, n_ch, GW], f32, tag="M")
                            nc.vector.tensor_tensor(
                                mtile[:],
                                iota[:].unsqueeze(1).broadcast_to([128, n_ch, GW]),
                                dl[:, :n_ch].unsqueeze(2).broadcast_to(
                                    [128, n_ch, GW]),
                                op=mybir.AluOpType.is_equal)
                            nc.vector.tensor_tensor(
                                mtile[:], mtile[:],
                                vl[:, :n_ch].unsqueeze(2).broadcast_to(
                                    [128, n_ch, GW]),
                                op=mybir.AluOpType.mult)

                            # gathers
                            gts = []
                            for (c_off, n_sl, r) in calls:
                                gt = g_p.tile([128, CALL_SLOTS // 128, D], f32,
                                              tag="g")
                                base = r << 15
                                rows_r = min(32768, n_src - base)
                                nc.gpsimd.dma_gather(
                                    gt[:, : n_sl // 128, :],
                                    table.ap()[base:base + rows_r, :],
                                    isb[:, c_off // 16:(c_off + n_sl) // 16],
                                    n_sl, n_sl, D)
                                gts.append((gt, c_off // 128, n_sl // 128))

                            # segment matmuls
                            psum = ps_p.tile([128, 128], f32, tag="seg")
                            for k, (g, st, sp) in enumerate(chunks):
                                for (gt, k0, nk) in gts:
                                    if k0 <= k < k0 + nk:
                                        rhs = gt[:, k - k0, :]
                                        break
                                nc.tensor.matmul(
                                    psum[GW * g:GW * (g + 1), :],
                                    mtile[:, k, :], rhs, start=st, stop=sp)

                            # evict + outputs + stats
                            rows = min(128, nds - 128 * t)
                            ub = ev_p.tile([128, 128], f32, tag="ub")
                            nc.vector.tensor_copy(ub[:], psum[:])
                            nc.sync.dma_start(
                                outs[(side, b)].ap()[128 * t:128 * t + rows, :],
                                ub[:rows, :])
                            nc.vector.tensor_add(
                                mean_sl[:, 128 * t:128 * (t + 1)],
                                mean_sl[:, 128 * t:128 * (t + 1)], ub[:])
                            sq = ev_p.tile([128, 128], f32, tag="sq")
                            if rows < 128:
                                nc.vector.memset(sq[:], 0.0)
                            nc.vector.tensor_tensor(
                                sq[:rows, :], ub[:rows, :], ub[:rows, :],
                                op=mybir.AluOpType.mult)
                            nc.tensor.matmul(
                                ssq[:, ssq_col:ssq_col + 1], sq[:], ones_col[:],
                                start=(t == 0), stop=(t == T_ - 1))

                            slot_off += tile_slots
                            chunk_off += n_ch

                # ssq -> collective
                ssq_sb = slab_p.tile([128, 8], f32)
                nc.vector.tensor_copy(ssq_sb[:], ssq[:])
                nc.sync.dma_start(cc_in.ap()[:, :], ssq_sb[:])

            nc.gpsimd.collective_compute(
                "AllReduce", mybir.AluOpType.add,
                replica_groups=[list(range(NCORES))],
                ins=[cc_in.ap().opt()], outs=[cc_out.ap().opt()])

            # ---------- phase 2: inv-norm + l2n + heads ----------
            with tc.tile_pool(name="f", bufs=4) as f_p, \
                 tc.tile_pool(name="ps2", bufs=2, space="PSUM") as ps2_p, \
                 tc.tile_pool(name="bc", bufs=1) as bc_p:

                ssq_g = f_p.tile([128, 8], f32, tag="ssqg")
                nc.sync.dma_start(ssq_g[:], cc_out.ap()[:, :])
                inv = bc_p.tile([128, 8], f32)
                nc.vector.tensor_scalar_max(inv[:], ssq_g[:], EPS2)
                nc.scalar.activation(inv[:], inv[:],
                                     mybir.ActivationFunctionType.Sqrt)
                nc.vector.reciprocal(inv[:], inv[:])

                ident = bc_p.tile([128, 128], f32)
                from concourse.masks import make_identity
                make_identity(nc, ident[:])

                bcs = []
                for j in range(8):
                    tp = ps2_p.tile([1, 128], f32, tag="tp")
                    nc.tensor.transpose(tp[:], inv[:, j:j + 1], ident[:])
                    rowb = f_p.tile([1, 128], f32, tag="rowb")
                    nc.vector.tensor_copy(rowb[:], tp[:])
                    bp = ps2_p.tile([128, 128], f32, tag="bp")
                    nc.tensor.matmul(bp[:], ones_row[:], rowb[:],
                                     start=True, stop=True)
                    bct = bc_p.tile([128, 128], f32, tag=f"bc{j}")
                    nc.vector.tensor_copy(bct[:], bp[:])
                    bcs.append(bct)

                # l2n outputs (re-read u_b/i_b tiles, scale along d)
                for si, (side, nds, T) in enumerate([("u", USH, TU),
                                                     ("i", ISH, TI)]):
                    for b in range(4):
                        lkey = ("lu", b) if side == "u" else ("li", b)
                        for t in range(T):
                            rows = min(128, nds - 128 * t)
                            rb = f_p.tile([128, 128], f32, tag="rb")
                            nc.sync.dma_start(
                                rb[:rows, :],
                                outs[(side, b)].ap()[128 * t:128 * t + rows, :])
                            nc.vector.tensor_tensor(
                                rb[:rows, :], rb[:rows, :], bcs[b + 4 * si][:rows, :],
                                op=mybir.AluOpType.mult)
                            nc.sync.dma_start(
                                outs[lkey].ap()[128 * t:128 * t + rows, :],
                                rb[:rows, :])

                # heads: sigmoid(mean/4 @ W)
                for side, nds, T, mean_sl, w_sb, okey in [
                    ("u", USH, TU, mean_u, uw_sb, "user_out"),
                    ("i", ISH, TI, mean_i, iw_sb, "item_out"),
                ]:
                    nc.vector.tensor_scalar_mul(mean_sl[:], mean_sl[:], 0.25)
                    for t in range(T):
                        rows = min(128, nds - 128 * t)
                        tp2 = ps2_p.tile([128, 128], f32, tag="tp2")
                        nc.tensor.transpose(
                            tp2[:], mean_sl[:, 128 * t:128 * (t + 1)], ident[:])
                        mT = f_p.tile([128, 128], f32, tag="mT")
                        nc.vector.tensor_copy(mT[:], tp2[:])
                        op = ps2_p.tile([128, 128], f32, tag="op")
                        nc.tensor.matmul(op[:], mT[:], w_sb[:],
                                         start=True, stop=True)
                        ob = f_p.tile([128, 128], f32, tag="ob")
                        nc.scalar.activation(ob[:], op[:],
                                             mybir.ActivationFunctionType.Sigmoid)
                        nc.sync.dma_start(
                            outs[okey].ap()[128 * t:128 * t + rows, :],
                            ob[:rows, :])

    nc.compile()
    return nc


def kernel(**inputs):
    from concourse.bass_utils import run_bass_kernel_spmd

    per_core, layouts = _host_prep(inputs)

    nc = _build_bass(layouts)

    iota64 = np.tile(np.arange(GW, dtype=np.float32), (128, 1))
    in_maps = []
    for c in range(NCORES):
        m = {
            "ii_embed0": np.asarray(inputs["ii_embed0"], np.float32),
            "ii_embed1": np.asarray(inputs["ii_embed1"], np.float32),
            "ii_embed2": np.asarray(inputs["ii_embed2"], np.float32),
            "item_embedding": np.asarray(inputs["item_embedding"], np.float32),
            "uu_embed0": np.asarray(inputs["uu_embed0"], np.float32),
            "uu_embed1": np.asarray(inputs["uu_embed1"], np.float32),
            "uu_embed2": np.asarray(inputs["uu_embed2"], np.float32),
            "user_embedding": np.asarray(inputs["user_embedding"], np.float32),
            "u_w": np.asarray(inputs["u_w"], np.float32),
            "i_w": np.asarray(inputs["i_w"], np.float32),
            "iota64": iota64,
        }
        for (side, b), (idx16, dloc, vals, _, _, _) in per_core[c].items():
            m[f"idx_{side}{b}"] = idx16
            m[f"dloc_{side}{b}"] = dloc
            m[f"val_{side}{b}"] = vals
        in_maps.append(m)

    trace = os.environ.get("KERNEL_TRACE", "1") == "1"
    if trace:
        try:
            sys.path.insert(0, os.path.dirname(os.path.abspath(__file__)))
            import ntff_hook

            ntff_hook.install()
        except Exception:
            trace = False
    try:
        res = run_bass_kernel_spmd(nc, in_maps, core_ids=list(range(NCORES)),
                                   trace=trace)
    except Exception:
        if not trace:
            raise
        res = run_bass_kernel_spmd(nc, in_maps, core_ids=list(range(NCORES)),
                                   trace=False)
    kernel._last_exec_ns = res.exec_time_ns

    def cat(key, n):
        return np.concatenate([res.results[c][key][:n // NCORES]
                               for c in range(NCORES)], axis=0)

    u = [cat(f"u{b}", NU) for b in range(4)]
    i_ = [cat(f"i{b}", NI) for b in range(4)]
    lu = np.stack([cat(f"lu{b}", NU) for b in range(4)], axis=0)
    li = np.stack([cat(f"li{b}", NI) for b in range(4)], axis=0)
    user_out = cat("user_out", NU)
    item_out = cat("item_out", NI)
    return (user_out, item_out, lu, li,
            u[0], i_[0], u[1], i_[1], u[2], i_[2])


kernel._last_exec_ns = None
